# revision 24
# baseline (speedup 1.0000x reference)
"""Trainium2 Bass kernel for nn_Attention_2 (gnn_message_passing).

Pure data parallel over the batch/node dim B=32768: 8 NeuronCores each
process 4096 rows. Per 256-row super-tile, the per-head softmax/gate
pipeline runs in a transposed layout ((h,j) on partitions, b on free dim)
so every reduction is a TensorEngine matmul against tiny host-built bf16
constants; the memory-dominant aggregation over neighbors is 32 matmuls
per 128-row half with the per-row weights as a 4-live-column stationary
operand and host-pre-cast bf16 context as the moving operand, accumulating
straight into PSUM.

Key optimizations over the naive structure (287us -> ~157us):
- context, source_distance, all constants and the output travel as bf16
  (half the HBM traffic; fp32 upcast on the host), and the whole chain's
  matmuls are bf16 so their weight loads hit the fast-weight-load path;
- the softmax/gate chain has ~5 serial PE<->ACT/DVE round trips at ~2us
  apiece (semaphore propagation dominates), far more than one super-tile's
  aggregation work, so each chain is spread across THREE loop iterations
  (A: MM1/exp/MM2; B1: recip/MM3/wmult/MM4; B2: tanh/stt/MM5/region
  writes) with the aggregation matmuls of older super-tiles drained into
  every gap of the PE stream;
- the tile scheduler orders by simulated readiness, which would cluster
  the chain matmuls into one sparse window per iteration and let HAM
  rethrottle the PE — so every instruction is pinned to a per-iteration
  time grid (tile_set_cur_wait/tile_wait_until) that staggers aggregation
  batches around fixed chain-matmul slots;
- simi = exp(-d^2/2) for the whole shard is precomputed, staggered across
  the early slots; DMA triggers are split sync=inputs / gpsimd=outputs
  (epilogue outputs on the by-then-idle sync) so context descriptor
  generation never blocks, and 5 context buffers give the DMA queues ~2
  iterations of trigger slack to stay fed.
"""

import sys

for _p in ("/opt/trn_rl_repo", "/root/.axon_site/_ro/trn_rl_repo"):
    if _p not in sys.path:
        sys.path.insert(0, _p)

from contextlib import ExitStack

import numpy as np

import concourse.bass as bass
import concourse.mybir as mybir
import concourse.tile as tile
from concourse import bacc
from concourse.bass_utils import run_bass_kernel_spmd

# Problem shape (hardcoded; kernel.py must be self-contained)
B, K, D, H = 32768, 32, 192, 4
NCORES = 8
ROWS = B // NCORES          # 4096 rows per core
P = 128                     # partitions / rows per tile
NT = ROWS // P              # 32 tiles per core
G = 4                       # rows per aggregation block (G*K == P)
NB = P // G                 # 32 blocks per tile
HK = H * K                  # 128
ST = 2                      # tiles per super-tile (256-row softmax/gate chain)
SP = ST * P

F32 = mybir.dt.float32
BF16 = mybir.dt.bfloat16
REGW = NB * (P + G)  # 4224: aggregation stationary-weight region width

# packed-constant column offsets: c_all (f32) and c_allb (bf16)
C_BIAS, C_GBH, C_MASK = 0, 1, 2
CW = 130
B_KERN, B_BLK, B_E4, B_GD, B_HG = 0, 128, 132, 260, 388
CWB = 516

_CACHE: dict = {}


def build_program(nt: int = NT):
    rows = nt * P
    nc = bacc.Bacc("TRN2", target_bir_lowering=False, debug=False, num_devices=NCORES)

    # Host-pretransposed inputs: sd as [K, rows] and ctx (bf16) as
    # [P, nt*NB*D] with ctx_host[p, (t, j, d)] = context[b0(t) + 4j + p//K,
    # p%K, d] — so every per-tile DMA reads one contiguous run per partition.
    F32R = mybir.dt.float32r
    sd_d = nc.dram_tensor("sd", [K, rows], BF16, kind="ExternalInput").ap()
    ctx_d = nc.dram_tensor("ctx", [P, nt * NB * D], BF16, kind="ExternalInput").ap()
    call_d = nc.dram_tensor("c_all", [P, CW], F32R, kind="ExternalInput").ap()
    callb_d = nc.dram_tensor("c_allb", [P, CWB], BF16, kind="ExternalInput").ap()
    out_d = nc.dram_tensor("out", [D, rows], BF16, kind="ExternalOutput").ap()

    with tile.TileContext(nc) as tc, ExitStack() as ctx:
        consts = ctx.enter_context(tc.tile_pool(name="consts", bufs=1))
        ctbp = ctx.enter_context(tc.tile_pool(name="ctbp", bufs=5))
        smallp = ctx.enter_context(tc.tile_pool(name="smallp", bufs=10))
        outp = ctx.enter_context(tc.tile_pool(name="outp", bufs=4))
        ps_mm = ctx.enter_context(tc.tile_pool(name="ps_mm", bufs=4, space="PSUM"))
        ps_s = ctx.enter_context(tc.tile_pool(name="ps_s", bufs=2, space="PSUM"))
        ps_out = ctx.enter_context(tc.tile_pool(name="ps_out", bufs=2, space="PSUM"))
        ctbp_bufs_note = None  # (kept name stable)

        # Dense masked per-row aggregation weights: wc[p, 4j+i] is row
        # (4j+i)'s weight for k=p%K when p//K==i, else 0 — used as the tiny
        # 4-column MOVING operand while the bf16 context rides the
        # fast-weight-load stationary path (2 rows/cycle for 128-col loads).
        wcp = ctx.enter_context(tc.tile_pool(name="wcp", bufs=4))

        # input DMAs all on sync, consts/sd first so their descriptors are
        # processed before the deep context prefetch queue
        c_all = consts.tile([P, CW], F32)
        nc.sync.dma_start(c_all[:], call_d.bitcast(F32))
        c_allb = consts.tile([P, CWB], BF16)
        nc.sync.dma_start(c_allb[:], callb_d)
        sd_all = consts.tile([K, rows], BF16)
        nc.sync.dma_start(sd_all[:], sd_d)

        c_kern = c_allb[0:K, B_KERN:B_KERN + HK]
        c_bias = c_all[:, C_BIAS:C_BIAS + 1]
        c_blk = c_allb[:, B_BLK:B_BLK + H]
        c_e4 = c_allb[0:H, B_E4:B_E4 + HK]
        c_gd = c_allb[:, B_GD:B_GD + HK]
        c_gbh = c_all[:, C_GBH:C_GBH + 1]
        c_hg = c_allb[:, B_HG:B_HG + P]
        c_mask = c_all[:, C_MASK:C_MASK + P]

        # simi = exp(-0.5 sd^2) for the entire shard, precomputed in SP-sized
        # chunks during the initial context-DMA wait (PE is idle then anyway)
        simi_all = consts.tile([K, rows], BF16, name="simi_all")
        TICK0 = 14.0e-3
        for t in range(0, nt, ST):
            # chunk for slot n floored at slot n-2 so the precompute spreads
            # across the pipeline instead of stalling the first iterations
            with tc.tile_wait_until(max(0, t // ST - 2) * TICK0):
                sq = smallp.tile([K, SP], F32, tag="sm")
                nc.vector.tensor_mul(sq[:], sd_all[:, t * P:t * P + SP],
                                     sd_all[:, t * P:t * P + SP])
                nc.scalar.activation(simi_all[:, t * P:t * P + SP], sq[:],
                                     mybir.ActivationFunctionType.Exp,
                                     scale=-0.5)

        def agg_gen(wcs, ctb, r0):
            # Deferred aggregation for one super-tile: per block of 4 rows,
            # two matmuls with the context chunk as the (FWL) stationary
            # operand and the 4 masked weight columns moving; output lands
            # [d, row]-transposed in one PSUM bank per super-tile.
            out_ps = ps_out.tile([P, 4 * P], F32, tag="outps")
            for hh in range(ST):
                wc = wcs[hh]
                base = hh * 2 * P
                cb = hh * NB * D
                for j in range(NB):
                    def mm_a(j=j, wc=wc, base=base, cb=cb):
                        nc.tensor.matmul(
                            out_ps[:, base + G * j:base + G * j + G],
                            lhsT=ctb[:, cb + j * D:cb + j * D + P],
                            rhs=wc[:, G * j:G * j + G],
                            start=True, stop=True)
                    yield mm_a

                    def mm_b(j=j, wc=wc, base=base, cb=cb):
                        nc.tensor.matmul(
                            out_ps[0:D - P, base + P + G * j:base + P + G * j + G],
                            lhsT=ctb[:, cb + j * D + P:cb + j * D + D],
                            rhs=wc[:, G * j:G * j + G],
                            start=True, stop=True)
                    yield mm_b
            yield (out_ps, r0)

        def drain(g, n):
            got = None
            for _ in range(n):
                op = next(g, None)
                if op is None:
                    return got
                if callable(op):
                    op()
                else:
                    got = op
            return got

        def emit_fins(fin, eng=None):
            # PSUM->SBUF copies (bf16 cast) + output DMAs, issued after the
            # chain's DVE ops so they never delay the reciprocal path. In the
            # steady state the DMA triggers ride gpsimd (SWDGE) so no HWDGE
            # engine ever blocks waiting on the copies; the epilogue's go to
            # the by-then-idle sync engine to avoid SWDGE's long drain tail.
            out_ps, r0 = fin
            for hh in range(ST):
                base = hh * 2 * P
                r0h = r0 + hh * P
                sb_a = outp.tile([P, P], BF16)
                nc.vector.tensor_copy(sb_a[:], out_ps[:, base:base + P])
                sb_b = outp.tile([D - P, P], BF16)
                nc.vector.tensor_copy(sb_b[:],
                                      out_ps[0:D - P, base + P:base + 2 * P])
                (eng or nc.gpsimd).dma_start(out_d[0:P, r0h:r0h + P], sb_a[:])
                (eng or nc.gpsimd).dma_start(out_d[P:D, r0h:r0h + P], sb_b[:])

        # pipeline state
        pending = iter(())      # aggregation stream (3 super-tiles back)
        stA = None              # (t, s_ps, p_t): stage A output, for B1
        stB = None              # (t, w_t, gl_ps): stage B1 output, for B2
        ctbs = {}

        # The tile scheduler emits instructions in simulated-readiness order,
        # not program order — left alone it runs all 64 aggregation matmuls
        # (ready at iteration start) first and clusters the latency-bound
        # chain matmuls into one sparse window, which HAM-rethrottles the PE
        # every iteration. Pin everything to a per-iteration time grid whose
        # tick exceeds any simulated iteration length, staggering the
        # aggregation batches so the chain matmuls land in between and the PE
        # stream stays dense.
        TICK = 14.0e-3          # ms per iteration slot (sim-time only)
        it = 0

        def wait(off):
            return tc.tile_wait_until(it * TICK + off * 1e-3)

        assert nt % ST == 0
        for t in range(0, nt, ST):
            tc.tile_set_cur_wait(it * TICK)
            # chunk-major bf16 context super-tile, one DMA
            ctb = ctbp.tile([P, ST * NB * D], BF16)
            nc.sync.dma_start(ctb[:], ctx_d[:, t * NB * D:(t + ST) * NB * D])
            ctbs[t] = ctb

            # B2 head for t-4 and B1 head for t-2: their inputs are old, so
            # ACT/DVE start right at the slot base
            if stB is not None:
                bt2, w_b, gl_b = stB
                th = smallp.tile([HK, SP], F32, tag="sm")
                nc.scalar.activation(th[:], gl_b[:],
                                     mybir.ActivationFunctionType.Tanh,
                                     bias=c_gbh, scale=0.5)
                gated2 = smallp.tile([HK, SP], BF16, tag="sm")
                nc.vector.scalar_tensor_tensor(
                    out=gated2[:], in0=th[:], scalar=1.0, in1=w_b[:],
                    op0=mybir.AluOpType.add, op1=mybir.AluOpType.mult)
            if stA is not None:
                bt1, s_a, p_a = stA
                rs32 = smallp.tile([H, SP], F32, tag="sm")
                nc.vector.reciprocal_approx_fast(out=rs32[:], in_=s_a[:])
                rs = smallp.tile([H, SP], BF16, tag="sm")
                nc.vector.tensor_copy(rs[:], rs32[:])

            drain(pending, 28)
            if stA is not None:
                with wait(1.4):
                    sbc_ps = ps_mm.tile([HK, SP], F32, tag="mm")
                    nc.tensor.matmul(sbc_ps[:], lhsT=c_e4, rhs=rs[:])
                    w_t = smallp.tile([HK, SP], BF16, tag="sm")
                    nc.vector.tensor_mul(w_t[:], p_a[:], sbc_ps[:])
            with wait(1.5):
                drain(pending, 20)
            agg_src = None
            if stB is not None:
                with wait(2.2):
                    wrep_ps = ps_mm.tile([P, SP], F32, tag="mm")
                    nc.tensor.matmul(wrep_ps[:], lhsT=c_hg, rhs=gated2[:])
                    wcs = []
                    for hh in range(ST):
                        wc = wcp.tile([P, P], BF16)
                        nc.vector.tensor_mul(wc[:],
                                             wrep_ps[:, hh * P:(hh + 1) * P],
                                             c_mask)
                        wcs.append(wc)
                agg_src = (wcs, ctbs.pop(bt2), bt2 * P)
            with wait(2.4):
                logits_ps = ps_mm.tile([HK, SP], F32, tag="mm")
                nc.tensor.matmul(logits_ps[:], lhsT=c_kern,
                                 rhs=simi_all[:, t * P:t * P + SP])
                p_t = smallp.tile([HK, SP], BF16, tag="sm")
                nc.scalar.activation(p_t[:], logits_ps[:],
                                     mybir.ActivationFunctionType.Exp,
                                     bias=c_bias)
            with wait(2.5):
                drain(pending, 24)
            stB_new = None
            if stA is not None:
                with wait(3.4):
                    gl_ps = ps_mm.tile([HK, SP], F32, tag="mm")
                    nc.tensor.matmul(gl_ps[:], lhsT=c_gd, rhs=w_t[:])
                stB_new = (bt1, w_t, gl_ps)
            with wait(3.5):
                drain(pending, 24)
            with wait(4.6):
                s_ps = ps_s.tile([H, SP], F32, tag="s")
                nc.tensor.matmul(s_ps[:], lhsT=c_blk, rhs=p_t[:])
            with wait(4.7):
                fin = drain(pending, 1 << 30)
                if fin:
                    emit_fins(fin)
            if agg_src is not None:
                pending = agg_gen(*agg_src)
            stA, stB = (t, s_ps, p_t), stB_new
            it += 1

        # ---- epilogue: flush the last two chain stages and all aggregation
        bt1, s_a, p_a = stA
        rs32 = smallp.tile([H, SP], F32, tag="sm")
        nc.vector.reciprocal_approx_fast(out=rs32[:], in_=s_a[:])
        rs = smallp.tile([H, SP], BF16, tag="sm")
        nc.vector.tensor_copy(rs[:], rs32[:])
        sbc_ps = ps_mm.tile([HK, SP], F32, tag="mm")
        nc.tensor.matmul(sbc_ps[:], lhsT=c_e4, rhs=rs[:])
        w_t = smallp.tile([HK, SP], BF16, tag="sm")
        nc.vector.tensor_mul(w_t[:], p_a[:], sbc_ps[:])
        gl_ps = ps_mm.tile([HK, SP], F32, tag="mm")
        nc.tensor.matmul(gl_ps[:], lhsT=c_gd, rhs=w_t[:])
        stB_last = (bt1, w_t, gl_ps)
        for st in (stB, stB_last):
            tc.tile_set_cur_wait(it * TICK)
            bt2, w_b, gl_b = st
            th = smallp.tile([HK, SP], F32, tag="sm")
            nc.scalar.activation(th[:], gl_b[:],
                                 mybir.ActivationFunctionType.Tanh,
                                 bias=c_gbh, scale=0.5)
            gated2 = smallp.tile([HK, SP], BF16, tag="sm")
            nc.vector.scalar_tensor_tensor(
                out=gated2[:], in0=th[:], scalar=1.0, in1=w_b[:],
                op0=mybir.AluOpType.add, op1=mybir.AluOpType.mult)
            with wait(1.0):
                drain(pending, 64)
            with wait(2.2):
                wrep_ps = ps_mm.tile([P, SP], F32, tag="mm")
                nc.tensor.matmul(wrep_ps[:], lhsT=c_hg, rhs=gated2[:])
                wcs = []
                for hh in range(ST):
                    wc = wcp.tile([P, P], BF16)
                    nc.vector.tensor_mul(wc[:],
                                         wrep_ps[:, hh * P:(hh + 1) * P],
                                         c_mask)
                    wcs.append(wc)
            with wait(2.4):
                fin = drain(pending, 1 << 30)
                if fin:
                    emit_fins(fin, eng=nc.sync)
            pending = agg_gen(wcs, ctbs.pop(bt2), bt2 * P)
            it += 1
        tc.tile_set_cur_wait(it * TICK)
        fin = drain(pending, 1 << 30)
        if fin:
            emit_fins(fin, eng=nc.sync)

    nc.compile()
    return nc


def _softmax(x):
    e = np.exp(x - x.max())
    return e / e.sum()


def build_consts(kernels, biases, gate_W, gate_b, gate_weights, gate_bias):
    f32 = np.float32
    kern_r = np.ascontiguousarray(kernels.transpose(1, 0, 2).reshape(K, HK)).astype(f32)
    hg = _softmax(np.asarray(gate_weights, np.float64) + np.asarray(gate_bias, np.float64))
    import ml_dtypes
    c_all = np.zeros((P, CW), f32)
    c_all[:, C_BIAS] = biases.reshape(HK)
    c_all[:, C_GBH] = 0.5 * np.tile(gate_b, H)
    c_all[:, C_MASK:C_MASK + P] = (
        np.arange(P)[:, None] // K == np.arange(P)[None, :] % G)
    c_allb = np.zeros((P, CWB), f32)
    c_allb[0:K, B_KERN:B_KERN + HK] = kern_r
    c_allb[:, B_BLK:B_BLK + H] = np.kron(np.eye(H), np.ones((K, 1)))
    c_allb[0:H, B_E4:B_E4 + HK] = np.kron(np.eye(H), np.ones((1, K)))
    c_allb[:, B_GD:B_GD + HK] = np.kron(np.eye(H), gate_W)
    c_allb[:, B_HG:B_HG + P] = np.kron((0.5 * hg)[:, None] @ np.ones((1, H)),
                                       np.eye(K))
    return c_all, c_allb.astype(ml_dtypes.bfloat16)


def run(inputs: dict, trace: bool = False, **kw):
    """inputs: full-size arrays keyed as in setup_inputs(). Returns (out, results)."""
    import ml_dtypes

    if "nc" not in _CACHE:
        _CACHE["nc"] = build_program()
    nc = _CACHE["nc"]

    sd = np.asarray(inputs["source_distance"],
                    np.float32).astype(ml_dtypes.bfloat16)
    ctx = np.asarray(inputs["context"], np.float32).astype(ml_dtypes.bfloat16)
    c_all, c_allb = build_consts(
        np.asarray(inputs["kernels"], np.float32),
        np.asarray(inputs["biases"], np.float32),
        np.asarray(inputs["gate_W"], np.float32),
        np.asarray(inputs["gate_b"], np.float32),
        np.asarray(inputs["gate_weights"], np.float32),
        np.asarray(inputs["gate_bias"], np.float32),
    )

    in_maps = []
    for c in range(NCORES):
        b0 = c * ROWS
        # host-side layout transforms so every device DMA run is long+contiguous
        sd_c = np.ascontiguousarray(sd[b0:b0 + ROWS].T)                    # [K, ROWS]
        ctx_c = np.ascontiguousarray(
            ctx[b0:b0 + ROWS].reshape(NT, NB, P, D).transpose(2, 0, 1, 3)
        ).reshape(P, NT * NB * D)
        in_maps.append({"sd": sd_c, "ctx": ctx_c, "c_all": c_all,
                        "c_allb": c_allb})

    results = run_bass_kernel_spmd(nc, in_maps, core_ids=list(range(NCORES)),
                                   trace=trace, **kw)
    out = np.concatenate(
        [results.results[c]["out"].T.astype(np.float32) for c in range(NCORES)],
        axis=0)
    return out, results


def kernel(**inputs) -> np.ndarray:
    out, _ = run(inputs)
    return out


# revision 25
# speedup vs baseline: 2.2041x; 2.2041x over previous
"""Trainium2 Bass kernel for nn_Attention_2 (gnn_message_passing).

Pure data parallel over the batch/node dim B=32768: 8 NeuronCores each
process 4096 rows. Per 256-row super-tile, the per-head softmax/gate
pipeline runs in a transposed layout ((h,j) on partitions, b on free dim)
so every reduction is a TensorEngine matmul against tiny host-built bf16
constants; the memory-dominant aggregation over neighbors is 32 matmuls
per 128-row half with the per-row weights as a 4-live-column stationary
operand and host-pre-cast bf16 context as the moving operand, accumulating
straight into PSUM.

Key optimizations over the naive structure (287us -> ~157us):
- context, source_distance, all constants and the output travel as bf16
  (half the HBM traffic; fp32 upcast on the host), and the whole chain's
  matmuls are bf16 so their weight loads hit the fast-weight-load path;
- the softmax/gate chain has ~5 serial PE<->ACT/DVE round trips at ~2us
  apiece (semaphore propagation dominates), far more than one super-tile's
  aggregation work, so each chain is spread across THREE loop iterations
  (A: MM1/exp/MM2; B1: recip/MM3/wmult/MM4; B2: tanh/stt/MM5/region
  writes) with the aggregation matmuls of older super-tiles drained into
  every gap of the PE stream;
- the tile scheduler orders by simulated readiness, which would cluster
  the chain matmuls into one sparse window per iteration and let HAM
  rethrottle the PE — so every instruction is pinned to a per-iteration
  time grid (tile_set_cur_wait/tile_wait_until) that staggers aggregation
  batches around fixed chain-matmul slots;
- simi = exp(-d^2/2) for the whole shard is precomputed, staggered across
  the early slots; DMA triggers are split sync=inputs / gpsimd=outputs
  (epilogue outputs on the by-then-idle sync) so context descriptor
  generation never blocks, and 5 context buffers give the DMA queues ~2
  iterations of trigger slack to stay fed.
"""

import sys

for _p in ("/opt/trn_rl_repo", "/root/.axon_site/_ro/trn_rl_repo"):
    if _p not in sys.path:
        sys.path.insert(0, _p)

from contextlib import ExitStack

import numpy as np

import concourse.bass as bass
import concourse.mybir as mybir
import concourse.tile as tile
from concourse import bacc
from concourse.bass_utils import run_bass_kernel_spmd

# Problem shape (hardcoded; kernel.py must be self-contained)
B, K, D, H = 32768, 32, 192, 4
NCORES = 8
ROWS = B // NCORES          # 4096 rows per core
P = 128                     # partitions / rows per tile
NT = ROWS // P              # 32 tiles per core
G = 4                       # rows per aggregation block (G*K == P)
NB = P // G                 # 32 blocks per tile
HK = H * K                  # 128
ST = 2                      # tiles per super-tile (256-row softmax/gate chain)
SP = ST * P

F32 = mybir.dt.float32
BF16 = mybir.dt.bfloat16
REGW = NB * (P + G)  # 4224: aggregation stationary-weight region width

# packed-constant column offsets: c_all (f32) and c_allb (bf16)
C_BIAS, C_GBH, C_MASK = 0, 1, 2
CW = 130
B_KERN, B_BLK, B_E4, B_GD, B_HG = 0, 128, 132, 260, 388
CWB = 516

_CACHE: dict = {}


def build_program(nt: int = NT):
    rows = nt * P
    nc = bacc.Bacc("TRN2", target_bir_lowering=False, debug=False, num_devices=NCORES)

    # Host-pretransposed inputs: sd as [K, rows] and ctx (bf16) as
    # [P, nt*NB*D] with ctx_host[p, (t, j, d)] = context[b0(t) + 4j + p//K,
    # p%K, d] — so every per-tile DMA reads one contiguous run per partition.
    F32R = mybir.dt.float32r
    sd_d = nc.dram_tensor("sd", [K, rows], BF16, kind="ExternalInput").ap()
    ctx_d = nc.dram_tensor("ctx", [P, nt * NB * D], BF16, kind="ExternalInput").ap()
    call_d = nc.dram_tensor("c_all", [P, CW], F32R, kind="ExternalInput").ap()
    callb_d = nc.dram_tensor("c_allb", [P, CWB], BF16, kind="ExternalInput").ap()
    out_d = nc.dram_tensor("out", [rows, D], BF16, kind="ExternalOutput").ap()

    with tile.TileContext(nc) as tc, ExitStack() as ctx:
        consts = ctx.enter_context(tc.tile_pool(name="consts", bufs=1))
        ctbp = ctx.enter_context(tc.tile_pool(name="ctbp", bufs=5))
        smallp = ctx.enter_context(tc.tile_pool(name="smallp", bufs=10))
        outp = ctx.enter_context(tc.tile_pool(name="outp", bufs=4))
        ps_mm = ctx.enter_context(tc.tile_pool(name="ps_mm", bufs=4, space="PSUM"))
        ps_s = ctx.enter_context(tc.tile_pool(name="ps_s", bufs=2, space="PSUM"))
        ps_out = ctx.enter_context(tc.tile_pool(name="ps_out", bufs=2, space="PSUM"))

        # Stationary-weight regions for the aggregation matmuls: 32 windows of
        # 128 bf16 columns spaced 132 apart; window j's only nonzero columns
        # are 4j..4j+3 (at col offset 136j), rewritten every tile. The rest
        # stays zero from the one-time memsets (f32-bitcast for 2x rate, split
        # across vector+gpsimd so they overlap the leading DMAs).
        regions = []
        for ri in range(4):
            reg = consts.tile([P, REGW], BF16, name=f"agg_region{ri}")
            regions.append(reg)
            eng = nc.vector if ri % 2 == 0 else nc.gpsimd
            eng.memset(reg[:].bitcast(F32), 0.0)

        # input DMAs all on sync, consts/sd first so their descriptors are
        # processed before the deep context prefetch queue
        c_all = consts.tile([P, CW], F32)
        nc.sync.dma_start(c_all[:], call_d.bitcast(F32))
        c_allb = consts.tile([P, CWB], BF16)
        nc.sync.dma_start(c_allb[:], callb_d)
        sd_all = consts.tile([K, rows], BF16)
        nc.sync.dma_start(sd_all[:], sd_d)

        c_kern = c_allb[0:K, B_KERN:B_KERN + HK]
        c_bias = c_all[:, C_BIAS:C_BIAS + 1]
        c_blk = c_allb[:, B_BLK:B_BLK + H]
        c_e4 = c_allb[0:H, B_E4:B_E4 + HK]
        c_gd = c_allb[:, B_GD:B_GD + HK]
        c_gbh = c_all[:, C_GBH:C_GBH + 1]
        c_hg = c_allb[:, B_HG:B_HG + P]
        c_mask = c_all[:, C_MASK:C_MASK + P]

        # simi = exp(-0.5 sd^2) for the entire shard, precomputed in SP-sized
        # chunks during the initial context-DMA wait (PE is idle then anyway)
        simi_all = consts.tile([K, rows], BF16, name="simi_all")
        TICK0 = 14.0e-3
        for t in range(0, nt, ST):
            # chunk for slot n floored at slot n-2 so the precompute spreads
            # across the pipeline instead of stalling the first iterations
            with tc.tile_wait_until(max(0, t // ST - 2) * TICK0):
                sq = smallp.tile([K, SP], F32, tag="sm")
                nc.vector.tensor_mul(sq[:], sd_all[:, t * P:t * P + SP],
                                     sd_all[:, t * P:t * P + SP])
                nc.scalar.activation(simi_all[:, t * P:t * P + SP], sq[:],
                                     mybir.ActivationFunctionType.Exp,
                                     scale=-0.5)

        def region_write_view(reg):
            # [128, 32, 4] view hitting cols 136j + i (the live columns of
            # window j, which starts at col 132j)
            return reg[:].rearrange("p (j x) -> p j x", x=G)[:, 0:REGW // G:(P + 2 * G) // G, :]

        mview = c_mask.rearrange("p (j x) -> p j x", x=G)

        def agg_gen(hregs, ctb, r0):
            # Deferred aggregation matmuls for one super-tile: per 128-row
            # half, 32 PSUM-accumulating matmuls (window j's stationary weight
            # has nonzeros only in out-partition columns 4j..4j+3).
            out_pss = []
            for hh in range(ST):
                reg = hregs[hh]
                out_ps = ps_out.tile([P, D], F32, tag="outps")
                out_pss.append(out_ps)
                for j in range(NB):
                    def mm(j=j, hh=hh, reg=reg, out_ps=out_ps):
                        nc.tensor.matmul(
                            out_ps[:],
                            lhsT=reg[:, (P + G) * j:(P + G) * j + P],
                            rhs=ctb[:, (hh * NB + j) * D:(hh * NB + j + 1) * D],
                            start=(j == 0), stop=(j == NB - 1))
                    yield mm
            yield (out_pss, r0)

        def drain(g, n):
            got = None
            for _ in range(n):
                op = next(g, None)
                if op is None:
                    return got
                if callable(op):
                    op()
                else:
                    got = op
            return got

        def emit_fins(fin, eng=None):
            # PSUM->SBUF copies (bf16 cast) + output DMAs, issued after the
            # chain's DVE ops so they never delay the reciprocal path. In the
            # steady state the DMA triggers ride gpsimd (SWDGE) so no HWDGE
            # engine ever blocks waiting on the copies; the epilogue's go to
            # the by-then-idle sync engine to avoid SWDGE's long drain tail.
            out_pss, r0 = fin
            for hh, out_ps in enumerate(out_pss):
                out_sb = outp.tile([P, D], BF16)
                nc.vector.tensor_copy(out_sb[:], out_ps[:])
                (eng or nc.gpsimd).dma_start(
                    out_d[r0 + hh * P:r0 + (hh + 1) * P, :], out_sb[:])

        # pipeline state
        pending = iter(())      # aggregation stream (3 super-tiles back)
        stA = None              # (t, s_ps, p_t): stage A output, for B1
        stB = None              # (t, w_t, gl_ps): stage B1 output, for B2
        ctbs = {}

        # The tile scheduler emits instructions in simulated-readiness order,
        # not program order — left alone it runs all 64 aggregation matmuls
        # (ready at iteration start) first and clusters the latency-bound
        # chain matmuls into one sparse window, which HAM-rethrottles the PE
        # every iteration. Pin everything to a per-iteration time grid whose
        # tick exceeds any simulated iteration length, staggering the
        # aggregation batches so the chain matmuls land in between and the PE
        # stream stays dense.
        TICK = 14.0e-3          # ms per iteration slot (sim-time only)
        it = 0

        def wait(off):
            return tc.tile_wait_until(it * TICK + off * 1e-3)

        assert nt % ST == 0
        for t in range(0, nt, ST):
            tc.tile_set_cur_wait(it * TICK)
            # chunk-major bf16 context super-tile, one DMA
            ctb = ctbp.tile([P, ST * NB * D], BF16)
            nc.sync.dma_start(ctb[:], ctx_d[:, t * NB * D:(t + ST) * NB * D])
            ctbs[t] = ctb

            # B2 head for t-4 and B1 head for t-2: their inputs are old, so
            # ACT/DVE start right at the slot base
            if stB is not None:
                bt2, w_b, gl_b = stB
                th = smallp.tile([HK, SP], F32, tag="sm")
                nc.scalar.activation(th[:], gl_b[:],
                                     mybir.ActivationFunctionType.Tanh,
                                     bias=c_gbh, scale=0.5)
                gated2 = smallp.tile([HK, SP], BF16, tag="sm")
                nc.vector.scalar_tensor_tensor(
                    out=gated2[:], in0=th[:], scalar=1.0, in1=w_b[:],
                    op0=mybir.AluOpType.add, op1=mybir.AluOpType.mult)
            if stA is not None:
                bt1, s_a, p_a = stA
                rs32 = smallp.tile([H, SP], F32, tag="sm")
                nc.vector.reciprocal_approx_fast(out=rs32[:], in_=s_a[:])
                rs = smallp.tile([H, SP], BF16, tag="sm")
                nc.vector.tensor_copy(rs[:], rs32[:])

            drain(pending, 14)
            if stA is not None:
                with wait(1.4):
                    sbc_ps = ps_mm.tile([HK, SP], F32, tag="mm")
                    nc.tensor.matmul(sbc_ps[:], lhsT=c_e4, rhs=rs[:])
                    w_t = smallp.tile([HK, SP], BF16, tag="sm")
                    nc.vector.tensor_mul(w_t[:], p_a[:], sbc_ps[:])
            with wait(1.5):
                drain(pending, 10)
            agg_src = None
            if stB is not None:
                with wait(2.2):
                    wrep_ps = ps_mm.tile([P, SP], F32, tag="mm")
                    nc.tensor.matmul(wrep_ps[:], lhsT=c_hg, rhs=gated2[:])
                    hregs = []
                    for hh in range(ST):
                        reg = regions[(bt2 + hh) % 4]
                        wview = wrep_ps[:, hh * P:(hh + 1) * P].rearrange(
                            "p (j x) -> p j x", x=G)
                        nc.vector.tensor_mul(region_write_view(reg), wview,
                                             mview)
                        hregs.append(reg)
                agg_src = (hregs, ctbs.pop(bt2), bt2 * P)
            with wait(2.4):
                logits_ps = ps_mm.tile([HK, SP], F32, tag="mm")
                nc.tensor.matmul(logits_ps[:], lhsT=c_kern,
                                 rhs=simi_all[:, t * P:t * P + SP])
                p_t = smallp.tile([HK, SP], BF16, tag="sm")
                nc.scalar.activation(p_t[:], logits_ps[:],
                                     mybir.ActivationFunctionType.Exp,
                                     bias=c_bias)
            with wait(2.5):
                drain(pending, 12)
            stB_new = None
            if stA is not None:
                with wait(3.4):
                    gl_ps = ps_mm.tile([HK, SP], F32, tag="mm")
                    nc.tensor.matmul(gl_ps[:], lhsT=c_gd, rhs=w_t[:])
                stB_new = (bt1, w_t, gl_ps)
            with wait(3.5):
                drain(pending, 12)
            with wait(4.6):
                s_ps = ps_s.tile([H, SP], F32, tag="s")
                nc.tensor.matmul(s_ps[:], lhsT=c_blk, rhs=p_t[:])
            with wait(4.7):
                fin = drain(pending, 1 << 30)
                if fin:
                    emit_fins(fin)
            if agg_src is not None:
                pending = agg_gen(*agg_src)
            stA, stB = (t, s_ps, p_t), stB_new
            it += 1

        # ---- epilogue: flush the last two chain stages and all aggregation
        bt1, s_a, p_a = stA
        rs32 = smallp.tile([H, SP], F32, tag="sm")
        nc.vector.reciprocal_approx_fast(out=rs32[:], in_=s_a[:])
        rs = smallp.tile([H, SP], BF16, tag="sm")
        nc.vector.tensor_copy(rs[:], rs32[:])
        sbc_ps = ps_mm.tile([HK, SP], F32, tag="mm")
        nc.tensor.matmul(sbc_ps[:], lhsT=c_e4, rhs=rs[:])
        w_t = smallp.tile([HK, SP], BF16, tag="sm")
        nc.vector.tensor_mul(w_t[:], p_a[:], sbc_ps[:])
        gl_ps = ps_mm.tile([HK, SP], F32, tag="mm")
        nc.tensor.matmul(gl_ps[:], lhsT=c_gd, rhs=w_t[:])
        stB_last = (bt1, w_t, gl_ps)
        for st in (stB, stB_last):
            tc.tile_set_cur_wait(it * TICK)
            bt2, w_b, gl_b = st
            th = smallp.tile([HK, SP], F32, tag="sm")
            nc.scalar.activation(th[:], gl_b[:],
                                 mybir.ActivationFunctionType.Tanh,
                                 bias=c_gbh, scale=0.5)
            gated2 = smallp.tile([HK, SP], BF16, tag="sm")
            nc.vector.scalar_tensor_tensor(
                out=gated2[:], in0=th[:], scalar=1.0, in1=w_b[:],
                op0=mybir.AluOpType.add, op1=mybir.AluOpType.mult)
            with wait(1.0):
                drain(pending, 32)
            with wait(2.2):
                wrep_ps = ps_mm.tile([P, SP], F32, tag="mm")
                nc.tensor.matmul(wrep_ps[:], lhsT=c_hg, rhs=gated2[:])
                hregs = []
                for hh in range(ST):
                    reg = regions[(bt2 + hh) % 4]
                    wview = wrep_ps[:, hh * P:(hh + 1) * P].rearrange(
                        "p (j x) -> p j x", x=G)
                    nc.vector.tensor_mul(region_write_view(reg), wview, mview)
                    hregs.append(reg)
            with wait(2.4):
                fin = drain(pending, 1 << 30)
                if fin:
                    emit_fins(fin, eng=nc.sync)
            pending = agg_gen(hregs, ctbs.pop(bt2), bt2 * P)
            it += 1
        tc.tile_set_cur_wait(it * TICK)
        fin = drain(pending, 1 << 30)
        if fin:
            emit_fins(fin, eng=nc.sync)

    nc.compile()
    return nc


def _softmax(x):
    e = np.exp(x - x.max())
    return e / e.sum()


def build_consts(kernels, biases, gate_W, gate_b, gate_weights, gate_bias):
    f32 = np.float32
    kern_r = np.ascontiguousarray(kernels.transpose(1, 0, 2).reshape(K, HK)).astype(f32)
    hg = _softmax(np.asarray(gate_weights, np.float64) + np.asarray(gate_bias, np.float64))
    import ml_dtypes
    c_all = np.zeros((P, CW), f32)
    c_all[:, C_BIAS] = biases.reshape(HK)
    c_all[:, C_GBH] = 0.5 * np.tile(gate_b, H)
    c_all[:, C_MASK:C_MASK + P] = (
        np.arange(P)[:, None] // K == np.arange(P)[None, :] % G)
    c_allb = np.zeros((P, CWB), f32)
    c_allb[0:K, B_KERN:B_KERN + HK] = kern_r
    c_allb[:, B_BLK:B_BLK + H] = np.kron(np.eye(H), np.ones((K, 1)))
    c_allb[0:H, B_E4:B_E4 + HK] = np.kron(np.eye(H), np.ones((1, K)))
    c_allb[:, B_GD:B_GD + HK] = np.kron(np.eye(H), gate_W)
    c_allb[:, B_HG:B_HG + P] = np.kron((0.5 * hg)[:, None] @ np.ones((1, H)),
                                       np.eye(K))
    return c_all, c_allb.astype(ml_dtypes.bfloat16)


def run(inputs: dict, trace: bool = False, **kw):
    """inputs: full-size arrays keyed as in setup_inputs(). Returns (out, results)."""
    import ml_dtypes

    if "nc" not in _CACHE:
        _CACHE["nc"] = build_program()
    nc = _CACHE["nc"]

    sd = np.asarray(inputs["source_distance"],
                    np.float32).astype(ml_dtypes.bfloat16)
    ctx = np.asarray(inputs["context"], np.float32).astype(ml_dtypes.bfloat16)
    c_all, c_allb = build_consts(
        np.asarray(inputs["kernels"], np.float32),
        np.asarray(inputs["biases"], np.float32),
        np.asarray(inputs["gate_W"], np.float32),
        np.asarray(inputs["gate_b"], np.float32),
        np.asarray(inputs["gate_weights"], np.float32),
        np.asarray(inputs["gate_bias"], np.float32),
    )

    in_maps = []
    for c in range(NCORES):
        b0 = c * ROWS
        # host-side layout transforms so every device DMA run is long+contiguous
        sd_c = np.ascontiguousarray(sd[b0:b0 + ROWS].T)                    # [K, ROWS]
        ctx_c = np.ascontiguousarray(
            ctx[b0:b0 + ROWS].reshape(NT, NB, P, D).transpose(2, 0, 1, 3)
        ).reshape(P, NT * NB * D)
        in_maps.append({"sd": sd_c, "ctx": ctx_c, "c_all": c_all,
                        "c_allb": c_allb})

    results = run_bass_kernel_spmd(nc, in_maps, core_ids=list(range(NCORES)),
                                   trace=trace, **kw)
    out = np.concatenate(
        [results.results[c]["out"].astype(np.float32) for c in range(NCORES)],
        axis=0)
    return out, results


def kernel(**inputs) -> np.ndarray:
    out, _ = run(inputs)
    return out
